# revision 10
# baseline (speedup 1.0000x reference)
"""Binary Matching Pursuit kernel for Trainium2 (8 cores, batch-sharded).

Math (per batch row, 11 serial steps; reference semantics):
    res = (2x - xr) @ W^T - lam*enc ; enc += onehot(argmax res)
    xr = top6_mask(enc @ W)

v2 reformulation (all decisions bit-match fp32 reference):
  - base' := (4x-1) @ W^T * 2^-17  ( = (2x @ W^T)/2^16 - colsum(W^T)/2^17 )
    computed ONCE on PE with a 3-way weight split (fp16+bf16+bf16, ~2^-27).
  - s := 2*xr - 1 in {-1,+1}; psum_s = s @ (-W^T)  => res' = psum_s*2^-17 - S
    where S := enc - base' (inhibition = 1.0 per win; res' values ~1e-3).
    u-matmul runs 2-way split (fp16+bf16, ~2^-19; xr is 6-sparse so error
    is far below WTA decision gaps).
  - WTA: vector.max (top8) + max_index -> winner index j*; S += (res'>=max).
  - z is NEVER computed as a matmul: each step gathers one weight row per
    batch row (native indirect DMA from DRAM, exact fp32) and accumulates it
    into a persistent PSUM bank via an identity matmul.
  - KWTA: vector.max on z_psum; threshold = midpoint of 6th/7th; next s via
    ACT sign(z - tmid); sT via PE transposes (lhsT for the next u-matmul).
  - encoded output = S_final - S_init (integer-exact).
"""

from contextlib import ExitStack

import numpy as np
import ml_dtypes

import concourse.bass as bass
import concourse.mybir as mybir
from concourse import masks
from concourse.tile import TileContext
from concourse.bass_utils import run_bass_kernel_spmd

B, IN_F, OUT_F = 4096, 512, 1024
N_CORES = 8
BC = B // N_CORES
P = 128
T_STEPS = 11
K_IN = 6
SCALE = 2.0 ** -17           # res' scale: base/2^16 with the (4x-1) fold

F32 = mybir.dt.float32
BF16 = mybir.dt.bfloat16
FP16 = mybir.dt.float16
U32 = mybir.dt.uint32
I16 = mybir.dt.int16
AX = mybir.AluOpType
ACTF = mybir.ActivationFunctionType
BF = ml_dtypes.bfloat16


def _split_hml(a):
    """f32 -> (h fp16, m bf16, l bf16); subnormal-prone fp16 values zeroed so
    host split matches any flush-to-zero hardware behavior."""
    h = a.astype(np.float16)
    h[np.abs(h.astype(np.float32)) < 6.2e-5] = 0
    r = a - h.astype(np.float32)
    m = r.astype(BF)
    l = (r - m.astype(np.float32)).astype(BF)
    return h, m, l


def _pack_k(a, nk):
    k, n = a.shape
    assert k == nk * P
    return np.ascontiguousarray(a.reshape(nk, P, n).transpose(1, 0, 2))


def _ts(i, s):
    return slice(i * s, (i + 1) * s)


def _legalize_waits(nc):
    """This walrus build accepts a single sync-wait per instruction; hoist
    extras onto same-engine NoOps inserted immediately before."""
    for f in nc.m.functions:
        for blk in f.blocks:
            new = []
            for inst in blk.instructions:
                si = inst.sync_info
                if (
                    si is not None
                    and si.on_wait
                    and len(si.on_wait) > 1
                    and type(inst).__name__ != "InstNoOp"
                ):
                    waits = list(si.on_wait)
                    for j, w in enumerate(waits[:-1]):
                        nop = mybir.InstNoOp(name=f"{inst.name}-wnop{j}", ins=[], outs=[])
                        nop.engine = inst.engine
                        nop.sync_info = mybir.SyncInfo(on_wait=[w], on_update=[])
                        new.append(nop)
                    inst.sync_info = mybir.SyncInfo(
                        on_wait=[waits[-1]], on_update=list(si.on_update)
                    )
                new.append(inst)
            blk.instructions = new
    return nc


def build_program(n_tiles=BC // P, n_steps=T_STEPS, repeat=1):
    nc = bass.Bass()

    NT = n_tiles
    # --- inputs ---
    d_x41 = [  # (4x-1)^T packed [128, 4, NT*128] in fp16 and bf16
        nc.declare_dram_parameter(f"x41_{d}", [P, 4, NT * P], t, isOutput=False)
        for d, t in (("h", FP16), ("b", BF16))
    ]
    # -W^T splits packed [128, 4, 1024]
    d_wTn = [
        nc.declare_dram_parameter(f"wTn_{s}", [P, 4, OUT_F], t, isOutput=False)
        for s, t in (("h", FP16), ("m", BF16), ("l", BF16))
    ]
    # gather source: raw weight rows [1024, 512] f32 in DRAM
    d_wrows = nc.declare_dram_parameter("w_rows", [OUT_F, IN_F], F32, isOutput=False)
    # --- outputs ---
    d_enc = nc.declare_dram_parameter("enc_out", [NT * P, OUT_F], F32, isOutput=True)
    d_xr = nc.declare_dram_parameter("xr_out", [NT * P, IN_F], F32, isOutput=True)

    with TileContext(nc) as tc, ExitStack() as ctx:
        const = ctx.enter_context(tc.tile_pool(name="const", bufs=1))
        id32 = const.tile([P, P], F32)
        masks.make_identity(nc, id32[:])
        idbf = const.tile([P, P], BF16)
        masks.make_identity(nc, idbf[:])

        wpool = ctx.enter_context(tc.tile_pool(name="wpool", bufs=1))
        x41 = []
        for i, d in enumerate("hb"):
            t = wpool.tile([P, 4 * NT * P], FP16 if d == "h" else BF16, name=f"x41{d}")
            nc.sync.dma_start(t[:], d_x41[i][:])
            x41.append(t)
        wTn = []
        for i, s in enumerate("hml"):
            t = wpool.tile([P, 4 * OUT_F], FP16 if s == "h" else BF16, name=f"wTn{s}")
            nc.sync.dma_start(t[:], d_wTn[i][:])
            wTn.append(t)
        half_neg = wpool.tile([P, 1], F32, name="half_neg")
        nc.vector.memset(half_neg[:], -0.5)

        state = ctx.enter_context(tc.tile_pool(name="state", bufs=1))
        S0, S, sTh, sTb = [], [], [], []
        for tb in range(NT):
            S0.append(state.tile([P, OUT_F], F32, tag=f"S0{tb}", name=f"S0{tb}"))
            S.append(state.tile([P, OUT_F], F32, tag=f"S{tb}", name=f"S{tb}"))
            sTh.append(state.tile([P, IN_F], FP16, tag=f"sTh{tb}", name=f"sTh{tb}"))
            sTb.append(state.tile([P, IN_F], BF16, tag=f"sTb{tb}", name=f"sTb{tb}"))

        scr = ctx.enter_context(tc.tile_pool(name="scr", bufs=2))
        p_s_pool = ctx.enter_context(tc.tile_pool(name="p_s", bufs=1, space="PSUM"))
        p_zT_pool = ctx.enter_context(tc.tile_pool(name="p_zT", bufs=2, space="PSUM"))
        p_z_pool = ctx.enter_context(tc.tile_pool(name="p_z", bufs=1, space="PSUM"))

        # persistent per-tile z accumulator banks
        p_z = [p_z_pool.tile([P, IN_F], F32, tag=f"p_z{tb}", name=f"p_z{tb}")
               for tb in range(NT)]

        for rep in range(repeat):
            # ---- init: S_init = -(4x-1)@wTn-splits * 2^-17 ... note wTn = -W^T
            # so psum_b = (4x-1)@(-W^T) and S_init = psum_b * 2^-17.
            for tb in range(NT):
                p_b = p_s_pool.tile([P, OUT_F], F32, tag="p_s", name="p_b")
                for k in range(4):
                    for s in range(3):
                        lhsT = (x41[0] if s == 0 else x41[1])[:, k * NT * P + tb * P : k * NT * P + (tb + 1) * P]
                        for nb in range(2):
                            nc.tensor.matmul(
                                p_b[:, _ts(nb, 512)],
                                lhsT=lhsT,
                                rhs=wTn[s][:, k * OUT_F + nb * 512 : k * OUT_F + (nb + 1) * 512],
                                start=(k == 0 and s == 0),
                                stop=(k == 3 and s == 2),
                            )
                nc.vector.tensor_scalar(
                    S0[tb][:], in0=p_b[:], scalar1=SCALE, scalar2=None, op0=AX.mult
                )
                nc.vector.tensor_copy(S[tb][:], S0[tb][:])
                nc.vector.memset(sTh[tb][:], -1.0)
                nc.vector.memset(sTb[tb][:], -1.0)

            # ---- serial MP steps ----
            for t in range(n_steps):
                last = t == n_steps - 1
                for tb in range(NT):
                    # u-matmul: psum_s = s @ (-W^T), 2-way split
                    p_s = p_s_pool.tile([P, OUT_F], F32, tag="p_s", name="p_s")
                    for k in range(4):
                        for s in range(2):
                            lhsT = (sTh[tb] if s == 0 else sTb[tb])[:, _ts(k, P)]
                            for nb in range(2):
                                nc.tensor.matmul(
                                    p_s[:, _ts(nb, 512)],
                                    lhsT=lhsT,
                                    rhs=wTn[s][:, k * OUT_F + nb * 512 : k * OUT_F + (nb + 1) * 512],
                                    start=(k == 0 and s == 0),
                                    stop=(k == 3 and s == 1),
                                )

                    # res' = psum_s * 2^-17 - S
                    res_s = scr.tile([P, OUT_F], F32, tag="res_s", name="res_s")
                    nc.vector.scalar_tensor_tensor(
                        res_s[:], in0=p_s[:], scalar=SCALE, in1=S[tb][:],
                        op0=AX.mult, op1=AX.subtract,
                    )
                    # WTA
                    m8 = scr.tile([P, 8], F32, tag="m8", name="m8")
                    nc.vector.max(m8[:], res_s[:])
                    nc.vector.scalar_tensor_tensor(
                        S[tb][:], in0=res_s[:], scalar=m8[:, 0:1], in1=S[tb][:],
                        op0=AX.is_ge, op1=AX.add,
                    )
                    idx8 = scr.tile([P, 8], U32, tag="idx8", name="idx8")
                    nc.vector.max_index(idx8[:], m8[:], res_s[:])

                    # gather winner weight rows (exact fp32) via indirect DMA,
                    # then accumulate into persistent z psum via identity mm
                    dz = scr.tile([P, IN_F], F32, tag="dz", name="dz")
                    nc.gpsimd.indirect_dma_start(
                        out=dz[:], out_offset=None, in_=d_wrows[:],
                        in_offset=bass.IndirectOffsetOnAxis(ap=idx8[:, 0:1], axis=0),
                    )
                    nc.tensor.matmul(
                        p_z[tb][:], lhsT=id32[:], rhs=dz[:],
                        start=(t == 0), stop=last,
                    )

                    # KWTA threshold: midpoint of 6th/7th largest of z
                    m8z = scr.tile([P, 8], F32, tag="m8z", name="m8z")
                    nc.vector.max(m8z[:], p_z[tb][:])
                    ntmid = scr.tile([P, 1], F32, tag="ntmid", name="ntmid")
                    nc.vector.scalar_tensor_tensor(
                        ntmid[:], in0=m8z[:, K_IN - 1 : K_IN],
                        scalar=m8z[:, K_IN : K_IN + 1], in1=half_neg[:],
                        op0=AX.add, op1=AX.mult,
                    )

                    if not last:
                        # s_next = sign(z - tmid) in {-1, +1}
                        sb = scr.tile([P, IN_F], BF16, tag="sb", name="sb")
                        nc.scalar.activation(sb[:], p_z[tb][:], ACTF.Sign,
                                             bias=ntmid[:, 0:1], scale=1.0)
                        p_sT = p_zT_pool.tile([P, IN_F], BF16, tag="p_zT", name="p_sT")
                        for k in range(4):
                            nc.tensor.matmul(
                                p_sT[:, _ts(k, P)], lhsT=sb[:, _ts(k, P)], rhs=idbf[:],
                                is_transpose=True, start=True, stop=True,
                            )
                        nc.scalar.copy(sTh[tb][:], p_sT[:])
                        nc.vector.tensor_copy(sTb[tb][:], p_sT[:])
                    else:
                        # outputs: xr = (z > tmid); enc = S - S_init
                        xr = scr.tile([P, IN_F], F32, tag="xr", name="xr")
                        nc.vector.tensor_scalar(
                            xr[:], in0=p_z[tb][:], scalar1=m8z[:, K_IN - 1 : K_IN],
                            scalar2=None, op0=AX.is_ge,
                        )
                        enc = scr.tile([P, OUT_F], F32, tag="enc", name="enc")
                        nc.vector.tensor_tensor(enc[:], S[tb][:], S0[tb][:],
                                                op=AX.subtract)
                        nc.sync.dma_start(d_enc[_ts(tb, P), :], enc[:])
                        nc.sync.dma_start(d_xr[_ts(tb, P), :], xr[:])

    return _legalize_waits(nc)


def make_in_maps(x, weight, n_tiles=BC // P):
    x = np.asarray(x, np.float32)
    weight = np.asarray(weight, np.float32)
    wT = np.ascontiguousarray(weight.T)               # [512, 1024]
    wTn_h, wTn_m, wTn_l = _split_hml(-wT)
    wTn = [_pack_k(a, 4) for a in (wTn_h, wTn_m, wTn_l)]
    bc = n_tiles * P
    in_maps = []
    for c in range(N_CORES):
        xs = x[c * BC : c * BC + bc]
        x41 = np.ascontiguousarray((4.0 * xs - 1.0).T)   # [512, bc], {-1, 3}
        x41p = _pack_k(x41, 4)
        m = {
            "x41_h": x41p.astype(np.float16),
            "x41_b": x41p.astype(BF),
            "wTn_h": wTn[0], "wTn_m": wTn[1], "wTn_l": wTn[2],
            "w_rows": np.ascontiguousarray(weight),
        }
        in_maps.append(m)
    return in_maps


_CACHE = {}


def run(x, weight, trace=False, **kw):
    if "nc" not in _CACHE:
        _CACHE["nc"] = build_program()
    res = run_bass_kernel_spmd(_CACHE["nc"], make_in_maps(x, weight),
                               list(range(N_CORES)), trace=trace, **kw)
    enc = np.concatenate([r["enc_out"] for r in res.results], 0)
    xr = np.concatenate([r["xr_out"] for r in res.results], 0)
    return (enc, xr), res


def kernel(x, weight):
    (enc, xr), _ = run(x, weight)
    return enc, xr


# revision 12
# speedup vs baseline: 1.0077x; 1.0077x over previous
"""Binary Matching Pursuit kernel for Trainium2 (8 cores, batch-sharded).

Math (per batch row, 11 serial steps; reference semantics):
    res = (2x - xr) @ W^T - lam*enc ; enc += onehot(argmax res)
    xr = top6_mask(enc @ W)

v2 reformulation (all decisions bit-match fp32 reference):
  - base' := (4x-1) @ W^T * 2^-17  ( = (2x @ W^T)/2^16 - colsum(W^T)/2^17 )
    computed ONCE on PE with a 3-way weight split (fp16+bf16+bf16, ~2^-27).
  - s := 2*xr - 1 in {-1,+1}; psum_s = s @ (-W^T)  => res' = psum_s*2^-17 - S
    where S := enc - base' (inhibition = 1.0 per win; res' values ~1e-3).
    u-matmul runs 2-way split (fp16+bf16, ~2^-19; xr is 6-sparse so error
    is far below WTA decision gaps).
  - WTA: vector.max (top8) + max_index -> winner index j*; S += (res'>=max).
  - z is NEVER computed as a matmul: each step gathers one weight row per
    batch row (native indirect DMA from DRAM, exact fp32) and accumulates it
    into a persistent PSUM bank via an identity matmul.
  - KWTA: vector.max on z_psum; threshold = midpoint of 6th/7th; next s via
    ACT sign(z - tmid); sT via PE transposes (lhsT for the next u-matmul).
  - encoded output = S_final - S_init (integer-exact).
"""

from contextlib import ExitStack

import numpy as np
import ml_dtypes

import concourse.bass as bass
import concourse.mybir as mybir
from concourse import masks
from concourse.tile import TileContext
from concourse.bass_utils import run_bass_kernel_spmd

B, IN_F, OUT_F = 4096, 512, 1024
N_CORES = 8
BC = B // N_CORES
P = 128
T_STEPS = 11
K_IN = 6
SCALE = 2.0 ** -17           # res' scale: base/2^16 with the (4x-1) fold

F32 = mybir.dt.float32
BF16 = mybir.dt.bfloat16
FP16 = mybir.dt.float16
U32 = mybir.dt.uint32
I16 = mybir.dt.int16
AX = mybir.AluOpType
ACTF = mybir.ActivationFunctionType
BF = ml_dtypes.bfloat16


def _split_hml(a):
    """f32 -> (h fp16, m bf16, l bf16); subnormal-prone fp16 values zeroed so
    host split matches any flush-to-zero hardware behavior."""
    h = a.astype(np.float16)
    h[np.abs(h.astype(np.float32)) < 6.2e-5] = 0
    r = a - h.astype(np.float32)
    m = r.astype(BF)
    l = (r - m.astype(np.float32)).astype(BF)
    return h, m, l


def _pack_k(a, nk):
    k, n = a.shape
    assert k == nk * P
    return np.ascontiguousarray(a.reshape(nk, P, n).transpose(1, 0, 2))


def _ts(i, s):
    return slice(i * s, (i + 1) * s)


def _legalize_waits(nc):
    """This walrus build accepts a single sync-wait per instruction; hoist
    extras onto same-engine NoOps inserted immediately before."""
    for f in nc.m.functions:
        for blk in f.blocks:
            new = []
            for inst in blk.instructions:
                si = inst.sync_info
                if (
                    si is not None
                    and si.on_wait
                    and len(si.on_wait) > 1
                    and type(inst).__name__ != "InstNoOp"
                ):
                    waits = list(si.on_wait)
                    for j, w in enumerate(waits[:-1]):
                        nop = mybir.InstNoOp(name=f"{inst.name}-wnop{j}", ins=[], outs=[])
                        nop.engine = inst.engine
                        nop.sync_info = mybir.SyncInfo(on_wait=[w], on_update=[])
                        new.append(nop)
                    inst.sync_info = mybir.SyncInfo(
                        on_wait=[waits[-1]], on_update=list(si.on_update)
                    )
                new.append(inst)
            blk.instructions = new
    return nc


def build_program(n_tiles=BC // P, n_steps=T_STEPS, repeat=1):
    nc = bass.Bass()

    NT = n_tiles
    # --- inputs ---
    d_x41 = [  # (4x-1)^T packed [128, 4, NT*128] in fp16 and bf16
        nc.declare_dram_parameter(f"x41_{d}", [P, 4, NT * P], t, isOutput=False)
        for d, t in (("h", FP16), ("b", BF16))
    ]
    # -W^T splits packed [128, 4, 1024]
    d_wTn = [
        nc.declare_dram_parameter(f"wTn_{s}", [P, 4, OUT_F], t, isOutput=False)
        for s, t in (("h", FP16), ("m", BF16), ("l", BF16))
    ]
    # gather source: raw weight rows [1024, 512] f32 in DRAM
    d_wrows = nc.declare_dram_parameter("w_rows", [OUT_F, IN_F], F32, isOutput=False)
    # --- outputs ---
    d_enc = nc.declare_dram_parameter("enc_out", [NT * P, OUT_F], F32, isOutput=True)
    d_xr = nc.declare_dram_parameter("xr_out", [NT * P, IN_F], F32, isOutput=True)

    with TileContext(nc) as tc, ExitStack() as ctx:
        const = ctx.enter_context(tc.tile_pool(name="const", bufs=1))
        id32 = const.tile([P, P], F32)
        masks.make_identity(nc, id32[:])
        idbf = const.tile([P, P], BF16)
        masks.make_identity(nc, idbf[:])

        wpool = ctx.enter_context(tc.tile_pool(name="wpool", bufs=1))
        x41 = []
        for i, d in enumerate("hb"):
            t = wpool.tile([P, 4 * NT * P], FP16 if d == "h" else BF16, name=f"x41{d}")
            nc.sync.dma_start(t[:], d_x41[i][:])
            x41.append(t)
        wTn = []
        for i, s in enumerate("hml"):
            t = wpool.tile([P, 4 * OUT_F], FP16 if s == "h" else BF16, name=f"wTn{s}")
            nc.sync.dma_start(t[:], d_wTn[i][:])
            wTn.append(t)
        half_neg = wpool.tile([P, 1], F32, name="half_neg")
        nc.vector.memset(half_neg[:], -0.5)

        state = ctx.enter_context(tc.tile_pool(name="state", bufs=1))
        S0, S, sTh, sTb = [], [], [], []
        for tb in range(NT):
            S0.append(state.tile([P, OUT_F], F32, tag=f"S0{tb}", name=f"S0{tb}"))
            S.append(state.tile([P, OUT_F], F32, tag=f"S{tb}", name=f"S{tb}"))
            sTh.append(state.tile([P, IN_F], FP16, tag=f"sTh{tb}", name=f"sTh{tb}"))
            sTb.append(state.tile([P, IN_F], BF16, tag=f"sTb{tb}", name=f"sTb{tb}"))

        scr = ctx.enter_context(tc.tile_pool(name="scr", bufs=3))
        p_s_pool = ctx.enter_context(tc.tile_pool(name="p_s", bufs=1, space="PSUM"))
        p_zT_pool = ctx.enter_context(tc.tile_pool(name="p_zT", bufs=2, space="PSUM"))
        p_z_pool = ctx.enter_context(tc.tile_pool(name="p_z", bufs=1, space="PSUM"))

        # persistent per-tile z accumulator banks
        p_z = [p_z_pool.tile([P, IN_F], F32, tag=f"p_z{tb}", name=f"p_z{tb}")
               for tb in range(NT)]

        for rep in range(repeat):
            # ---- init: S_init = -(4x-1)@wTn-splits * 2^-17 ... note wTn = -W^T
            # so psum_b = (4x-1)@(-W^T) and S_init = psum_b * 2^-17.
            for tb in range(NT):
                p_b = p_s_pool.tile([P, OUT_F], F32, tag="p_s", name="p_b")
                for k in range(4):
                    for s in range(3):
                        lhsT = (x41[0] if s == 0 else x41[1])[:, k * NT * P + tb * P : k * NT * P + (tb + 1) * P]
                        for nb in range(2):
                            nc.tensor.matmul(
                                p_b[:, _ts(nb, 512)],
                                lhsT=lhsT,
                                rhs=wTn[s][:, k * OUT_F + nb * 512 : k * OUT_F + (nb + 1) * 512],
                                start=(k == 0 and s == 0),
                                stop=(k == 3 and s == 2),
                            )
                nc.vector.tensor_scalar(
                    S0[tb][:], in0=p_b[:], scalar1=SCALE, scalar2=None, op0=AX.mult
                )
                nc.scalar.copy(S[tb][:], S0[tb][:])
                nc.vector.memset(sTh[tb][:], -1.0)
                nc.vector.memset(sTb[tb][:], -1.0)

            # ---- serial MP steps ----
            for t in range(n_steps):
                last = t == n_steps - 1
                for tb in range(NT):
                    # u-matmul: psum_s = s @ (-W^T), 2-way split
                    p_s = p_s_pool.tile([P, OUT_F], F32, tag="p_s", name="p_s")
                    for k in range(4):
                        for s in range(2):
                            lhsT = (sTh[tb] if s == 0 else sTb[tb])[:, _ts(k, P)]
                            for nb in range(2):
                                nc.tensor.matmul(
                                    p_s[:, _ts(nb, 512)],
                                    lhsT=lhsT,
                                    rhs=wTn[s][:, k * OUT_F + nb * 512 : k * OUT_F + (nb + 1) * 512],
                                    start=(k == 0 and s == 0),
                                    stop=(k == 3 and s == 1),
                                )
                    # res' = psum_s * 2^-17 - S
                    res_s = scr.tile([P, OUT_F], F32, tag="res_s", name="res_s")
                    nc.vector.scalar_tensor_tensor(
                        res_s[:], in0=p_s[:], scalar=SCALE, in1=S[tb][:],
                        op0=AX.mult, op1=AX.subtract,
                    )
                    # WTA
                    m8 = scr.tile([P, 8], F32, tag="m8", name="m8")
                    nc.vector.max(m8[:], res_s[:])
                    nc.vector.scalar_tensor_tensor(
                        S[tb][:], in0=res_s[:], scalar=m8[:, 0:1], in1=S[tb][:],
                        op0=AX.is_ge, op1=AX.add,
                    )
                    idx8 = scr.tile([P, 8], U32, tag="idx8", name="idx8")
                    nc.vector.max_index(idx8[:], m8[:], res_s[:])

                    # gather winner weight rows (exact fp32) via indirect DMA,
                    # then accumulate into persistent z psum via identity mm
                    dz = scr.tile([P, IN_F], F32, tag="dz", name="dz")
                    nc.gpsimd.indirect_dma_start(
                        out=dz[:], out_offset=None, in_=d_wrows[:],
                        in_offset=bass.IndirectOffsetOnAxis(ap=idx8[:, 0:1], axis=0),
                    )
                    nc.tensor.matmul(
                        p_z[tb][:], lhsT=id32[:], rhs=dz[:],
                        start=(t == 0), stop=last,
                    )

                    # KWTA threshold: midpoint of 6th/7th largest of z
                    m8z = scr.tile([P, 8], F32, tag="m8z", name="m8z")
                    nc.vector.max(m8z[:], p_z[tb][:])
                    ntmid = scr.tile([P, 1], F32, tag="ntmid", name="ntmid")
                    nc.vector.scalar_tensor_tensor(
                        ntmid[:], in0=m8z[:, K_IN - 1 : K_IN],
                        scalar=m8z[:, K_IN : K_IN + 1], in1=half_neg[:],
                        op0=AX.add, op1=AX.mult,
                    )

                    if not last:
                        # s_next = sign(z - tmid) in {-1, +1}
                        sb = scr.tile([P, IN_F], BF16, tag="sb", name="sb")
                        nc.scalar.activation(sb[:], p_z[tb][:], ACTF.Sign,
                                             bias=ntmid[:, 0:1], scale=1.0)
                        p_sT = p_zT_pool.tile([P, IN_F], BF16, tag="p_zT", name="p_sT")
                        for k in range(4):
                            nc.tensor.matmul(
                                p_sT[:, _ts(k, P)], lhsT=sb[:, _ts(k, P)], rhs=idbf[:],
                                is_transpose=True, start=True, stop=True,
                            )
                        nc.scalar.copy(sTh[tb][:], p_sT[:])
                        nc.scalar.copy(sTb[tb][:], p_sT[:])
                    else:
                        # outputs: xr = (z > tmid); enc = S - S_init
                        xr = scr.tile([P, IN_F], F32, tag="xr", name="xr")
                        nc.vector.tensor_scalar(
                            xr[:], in0=p_z[tb][:], scalar1=m8z[:, K_IN - 1 : K_IN],
                            scalar2=None, op0=AX.is_ge,
                        )
                        enc = scr.tile([P, OUT_F], F32, tag="enc", name="enc")
                        nc.vector.tensor_tensor(enc[:], S[tb][:], S0[tb][:],
                                                op=AX.subtract)
                        nc.sync.dma_start(d_enc[_ts(tb, P), :], enc[:])
                        nc.sync.dma_start(d_xr[_ts(tb, P), :], xr[:])

    return _legalize_waits(nc)


def make_in_maps(x, weight, n_tiles=BC // P):
    x = np.asarray(x, np.float32)
    weight = np.asarray(weight, np.float32)
    wT = np.ascontiguousarray(weight.T)               # [512, 1024]
    wTn_h, wTn_m, wTn_l = _split_hml(-wT)
    wTn = [_pack_k(a, 4) for a in (wTn_h, wTn_m, wTn_l)]
    bc = n_tiles * P
    in_maps = []
    for c in range(N_CORES):
        xs = x[c * BC : c * BC + bc]
        x41 = np.ascontiguousarray((4.0 * xs - 1.0).T)   # [512, bc], {-1, 3}
        x41p = _pack_k(x41, 4)
        m = {
            "x41_h": x41p.astype(np.float16),
            "x41_b": x41p.astype(BF),
            "wTn_h": wTn[0], "wTn_m": wTn[1], "wTn_l": wTn[2],
            "w_rows": np.ascontiguousarray(weight),
        }
        in_maps.append(m)
    return in_maps


_CACHE = {}


def run(x, weight, trace=False, **kw):
    if "nc" not in _CACHE:
        _CACHE["nc"] = build_program()
    res = run_bass_kernel_spmd(_CACHE["nc"], make_in_maps(x, weight),
                               list(range(N_CORES)), trace=trace, **kw)
    enc = np.concatenate([r["enc_out"] for r in res.results], 0)
    xr = np.concatenate([r["xr_out"] for r in res.results], 0)
    return (enc, xr), res


def kernel(x, weight):
    (enc, xr), _ = run(x, weight)
    return enc, xr


# revision 21
# speedup vs baseline: 1.0520x; 1.0440x over previous
"""Binary Matching Pursuit kernel for Trainium2 (8 cores, batch-sharded).

Math (per batch row, 11 serial steps; reference semantics):
    res = (2x - xr) @ W^T - lam*enc ; enc += onehot(argmax res)
    xr = top6_mask(enc @ W)

v2 reformulation (all decisions bit-match fp32 reference):
  - base' := (4x-1) @ W^T * 2^-17  ( = (2x @ W^T)/2^16 - colsum(W^T)/2^17 )
    computed ONCE on PE with a 3-way weight split (fp16+bf16+bf16, ~2^-27).
  - s := 2*xr - 1 in {-1,+1}; psum_s = s @ (-W^T)  => res' = psum_s*2^-17 - S
    where S := enc - base' (inhibition = 1.0 per win; res' values ~1e-3).
    u-matmul runs 2-way split (fp16+bf16, ~2^-19; xr is 6-sparse so error
    is far below WTA decision gaps).
  - WTA: vector.max (top8) + max_index -> winner index j*; S += (res'>=max).
  - z is NEVER computed as a matmul: each step gathers one weight row per
    batch row (native indirect DMA from DRAM, exact fp32) and accumulates it
    into a persistent PSUM bank via an identity matmul.
  - KWTA: vector.max on z_psum; threshold = midpoint of 6th/7th; next s via
    ACT sign(z - tmid); sT via PE transposes (lhsT for the next u-matmul).
  - encoded output = S_final - S_init (integer-exact).
"""

from contextlib import ExitStack

import numpy as np
import ml_dtypes

import concourse.bass as bass
import concourse.mybir as mybir
from concourse import masks
from concourse.tile import TileContext
from concourse.bass_utils import run_bass_kernel_spmd

B, IN_F, OUT_F = 4096, 512, 1024
N_CORES = 8
BC = B // N_CORES
P = 128
T_STEPS = 11
K_IN = 6
SCALE = 2.0 ** -17           # res' scale: base/2^16 with the (4x-1) fold

F32 = mybir.dt.float32
BF16 = mybir.dt.bfloat16
FP16 = mybir.dt.float16
U32 = mybir.dt.uint32
I16 = mybir.dt.int16
AX = mybir.AluOpType
ACTF = mybir.ActivationFunctionType
BF = ml_dtypes.bfloat16


def _split_hml(a):
    """f32 -> (h fp16, m bf16, l bf16); subnormal-prone fp16 values zeroed so
    host split matches any flush-to-zero hardware behavior."""
    h = a.astype(np.float16)
    h[np.abs(h.astype(np.float32)) < 6.2e-5] = 0
    r = a - h.astype(np.float32)
    m = r.astype(BF)
    l = (r - m.astype(np.float32)).astype(BF)
    return h, m, l


def _pack_k(a, nk):
    k, n = a.shape
    assert k == nk * P
    return np.ascontiguousarray(a.reshape(nk, P, n).transpose(1, 0, 2))


def _ts(i, s):
    return slice(i * s, (i + 1) * s)


def _legalize_waits(nc):
    """This walrus build accepts a single sync-wait per instruction; hoist
    extras onto same-engine NoOps inserted immediately before."""
    for f in nc.m.functions:
        for blk in f.blocks:
            new = []
            for inst in blk.instructions:
                si = inst.sync_info
                if (
                    si is not None
                    and si.on_wait
                    and len(si.on_wait) > 1
                    and type(inst).__name__ != "InstNoOp"
                ):
                    waits = list(si.on_wait)
                    for j, w in enumerate(waits[:-1]):
                        nop = mybir.InstNoOp(name=f"{inst.name}-wnop{j}", ins=[], outs=[])
                        nop.engine = inst.engine
                        nop.sync_info = mybir.SyncInfo(on_wait=[w], on_update=[])
                        new.append(nop)
                    inst.sync_info = mybir.SyncInfo(
                        on_wait=[waits[-1]], on_update=list(si.on_update)
                    )
                new.append(inst)
            blk.instructions = new
    return nc


def build_program(n_tiles=BC // P, n_steps=T_STEPS, repeat=1):
    nc = bass.Bass()

    NT = n_tiles
    # --- inputs ---
    d_x41 = [  # (4x-1)^T packed [128, 4, NT*128] in fp16 and bf16
        nc.declare_dram_parameter(f"x41_{d}", [P, 4, NT * P], t, isOutput=False)
        for d, t in (("h", FP16), ("b", BF16))
    ]
    # -W^T splits packed [128, 4, 1024]
    d_wTn = [
        nc.declare_dram_parameter(f"wTn_{s}", [P, 4, OUT_F], t, isOutput=False)
        for s, t in (("h", FP16), ("m", BF16), ("l", BF16))
    ]
    # gather source: raw weight rows [1024, 512] f32 in DRAM
    d_wrows = nc.declare_dram_parameter("w_rows", [OUT_F, IN_F], F32, isOutput=False)
    # --- outputs ---
    d_enc = nc.declare_dram_parameter("enc_out", [NT * P, OUT_F], F32, isOutput=True)
    d_xr = nc.declare_dram_parameter("xr_out", [NT * P, IN_F], F32, isOutput=True)

    with TileContext(nc) as tc, ExitStack() as ctx:
        const = ctx.enter_context(tc.tile_pool(name="const", bufs=1))
        id32 = const.tile([P, P], F32)
        masks.make_identity(nc, id32[:])
        idbf = const.tile([P, P], BF16)
        masks.make_identity(nc, idbf[:])

        wpool = ctx.enter_context(tc.tile_pool(name="wpool", bufs=1))
        x41 = []
        for i, d in enumerate("hb"):
            t = wpool.tile([P, 4 * NT * P], FP16 if d == "h" else BF16, name=f"x41{d}")
            nc.sync.dma_start(t[:], d_x41[i][:])
            x41.append(t)
        wTn = []
        for i, s in enumerate("hml"):
            t = wpool.tile([P, 4 * OUT_F], FP16 if s == "h" else BF16, name=f"wTn{s}")
            nc.sync.dma_start(t[:], d_wTn[i][:])
            wTn.append(t)
        half_neg = wpool.tile([P, 1], F32, name="half_neg")
        nc.vector.memset(half_neg[:], -0.5)

        state = ctx.enter_context(tc.tile_pool(name="state", bufs=1))
        S0, S, sTh, sTb = [], [], [], []
        for tb in range(NT):
            S0.append(state.tile([P, OUT_F], F32, tag=f"S0{tb}", name=f"S0{tb}"))
            S.append(state.tile([P, OUT_F], F32, tag=f"S{tb}", name=f"S{tb}"))
            sTh.append(state.tile([P, IN_F], FP16, tag=f"sTh{tb}", name=f"sTh{tb}"))
            sTb.append(state.tile([P, IN_F], BF16, tag=f"sTb{tb}", name=f"sTb{tb}"))

        scr = ctx.enter_context(tc.tile_pool(name="scr", bufs=3))
        p_s_pool = ctx.enter_context(tc.tile_pool(name="p_s", bufs=1, space="PSUM"))
        p_zT_pool = ctx.enter_context(tc.tile_pool(name="p_zT", bufs=2, space="PSUM"))
        p_z_pool = ctx.enter_context(tc.tile_pool(name="p_z", bufs=1, space="PSUM"))

        # persistent per-tile z accumulator banks
        p_z = [p_z_pool.tile([P, IN_F], F32, tag=f"p_z{tb}", name=f"p_z{tb}")
               for tb in range(NT)]

        for rep in range(repeat):
            # ---- init: S_init = -(4x-1)@wTn-splits * 2^-17 ... note wTn = -W^T
            # so psum_b = (4x-1)@(-W^T) and S_init = psum_b * 2^-17.
            for tb in range(NT):
                p_b = p_s_pool.tile([P, OUT_F], F32, tag="p_s", name="p_b")
                for k in range(4):
                    for s in range(3):
                        lhsT = (x41[0] if s == 0 else x41[1])[:, k * NT * P + tb * P : k * NT * P + (tb + 1) * P]
                        for nb in range(2):
                            nc.tensor.matmul(
                                p_b[:, _ts(nb, 512)],
                                lhsT=lhsT,
                                rhs=wTn[s][:, k * OUT_F + nb * 512 : k * OUT_F + (nb + 1) * 512],
                                start=(k == 0 and s == 0),
                                stop=(k == 3 and s == 2),
                            )
                nc.vector.tensor_scalar(
                    S0[tb][:], in0=p_b[:], scalar1=SCALE, scalar2=None, op0=AX.mult
                )
                nc.scalar.copy(S[tb][:], S0[tb][:])
                nc.vector.memset(sTh[tb][:], -1.0)
                nc.vector.memset(sTb[tb][:], -1.0)

            # ---- serial MP steps ----
            for t in range(n_steps):
                last = t == n_steps - 1
                for tb in range(NT):
                    # u-matmul: psum_s = s @ (-W^T), 2-way split
                    p_s = p_s_pool.tile([P, OUT_F], F32, tag="p_s", name="p_s")
                    for k in range(4):
                        for s in range(2):
                            lhsT = (sTh[tb] if s == 0 else sTb[tb])[:, _ts(k, P)]
                            for nb in range(2):
                                nc.tensor.matmul(
                                    p_s[:, _ts(nb, 512)],
                                    lhsT=lhsT,
                                    rhs=wTn[s][:, k * OUT_F + nb * 512 : k * OUT_F + (nb + 1) * 512],
                                    start=(k == 0 and s == 0),
                                    stop=(k == 3 and s == 1),
                                )
                    # res' = psum_s * 2^-17 - S
                    res_s = scr.tile([P, OUT_F], F32, tag="res_s", name="res_s")
                    nc.vector.scalar_tensor_tensor(
                        res_s[:], in0=p_s[:], scalar=SCALE, in1=S[tb][:],
                        op0=AX.mult, op1=AX.subtract,
                    )
                    # WTA
                    m8 = scr.tile([P, 8], F32, tag="m8", name="m8")
                    nc.vector.max(m8[:], res_s[:])
                    idx8 = scr.tile([P, 8], U32, tag="idx8", name="idx8")
                    nc.vector.max_index(idx8[:], m8[:], res_s[:])

                    # gather winner weight rows (exact fp32) via indirect DMA,
                    # then accumulate into persistent z psum via identity mm;
                    # the S-update runs in the gather's shadow
                    dz = scr.tile([P, IN_F], F32, tag="dz", name="dz")
                    nc.gpsimd.indirect_dma_start(
                        out=dz[:], out_offset=None, in_=d_wrows[:],
                        in_offset=bass.IndirectOffsetOnAxis(ap=idx8[:, 0:1], axis=0),
                    )
                    nc.vector.scalar_tensor_tensor(
                        S[tb][:], in0=res_s[:], scalar=m8[:, 0:1], in1=S[tb][:],
                        op0=AX.is_ge, op1=AX.add,
                    )
                    nc.tensor.matmul(
                        p_z[tb][:], lhsT=id32[:], rhs=dz[:],
                        start=(t == 0), stop=last,
                    )

                    # KWTA threshold: midpoint of 6th/7th largest of z
                    m8z = scr.tile([P, 8], F32, tag="m8z", name="m8z")
                    nc.vector.max(m8z[:], p_z[tb][:])
                    ntmid = scr.tile([P, 1], F32, tag="ntmid", name="ntmid")
                    nc.vector.scalar_tensor_tensor(
                        ntmid[:], in0=m8z[:, K_IN - 1 : K_IN],
                        scalar=m8z[:, K_IN : K_IN + 1], in1=half_neg[:],
                        op0=AX.add, op1=AX.mult,
                    )

                    if not last:
                        # s_next = sign(z - tmid) in {-1, +1}
                        sb = scr.tile([P, IN_F], BF16, tag="sb", name="sb")
                        nc.scalar.activation(sb[:], p_z[tb][:], ACTF.Sign,
                                             bias=ntmid[:, 0:1], scale=1.0)
                        p_sT = p_zT_pool.tile([P, IN_F], BF16, tag="p_zT", name="p_sT")
                        for k in range(4):
                            nc.tensor.matmul(
                                p_sT[:, _ts(k, P)], lhsT=sb[:, _ts(k, P)], rhs=idbf[:],
                                is_transpose=True, start=True, stop=True,
                            )
                        nc.scalar.copy(sTh[tb][:], p_sT[:])
                        nc.scalar.copy(sTb[tb][:], p_sT[:])
                    else:
                        # outputs: xr = (z > tmid); enc = S - S_init
                        xr = scr.tile([P, IN_F], F32, tag="xr", name="xr")
                        nc.vector.tensor_scalar(
                            xr[:], in0=p_z[tb][:], scalar1=m8z[:, K_IN - 1 : K_IN],
                            scalar2=None, op0=AX.is_ge,
                        )
                        enc = scr.tile([P, OUT_F], F32, tag="enc", name="enc")
                        nc.vector.tensor_tensor(enc[:], S[tb][:], S0[tb][:],
                                                op=AX.subtract)
                        nc.sync.dma_start(d_enc[_ts(tb, P), :], enc[:])
                        nc.sync.dma_start(d_xr[_ts(tb, P), :], xr[:])

    return _legalize_waits(nc)


def make_in_maps(x, weight, n_tiles=BC // P):
    x = np.asarray(x, np.float32)
    weight = np.asarray(weight, np.float32)
    wT = np.ascontiguousarray(weight.T)               # [512, 1024]
    wTn_h, wTn_m, wTn_l = _split_hml(-wT)
    wTn = [_pack_k(a, 4) for a in (wTn_h, wTn_m, wTn_l)]
    bc = n_tiles * P
    in_maps = []
    for c in range(N_CORES):
        xs = x[c * BC : c * BC + bc]
        x41 = np.ascontiguousarray((4.0 * xs - 1.0).T)   # [512, bc], {-1, 3}
        x41p = _pack_k(x41, 4)
        m = {
            "x41_h": x41p.astype(np.float16),
            "x41_b": x41p.astype(BF),
            "wTn_h": wTn[0], "wTn_m": wTn[1], "wTn_l": wTn[2],
            "w_rows": np.ascontiguousarray(weight),
        }
        in_maps.append(m)
    return in_maps


_CACHE = {}


def run(x, weight, trace=False, **kw):
    if "nc" not in _CACHE:
        _CACHE["nc"] = build_program()
    res = run_bass_kernel_spmd(_CACHE["nc"], make_in_maps(x, weight),
                               list(range(N_CORES)), trace=trace, **kw)
    enc = np.concatenate([r["enc_out"] for r in res.results], 0)
    xr = np.concatenate([r["xr_out"] for r in res.results], 0)
    return (enc, xr), res


def kernel(x, weight):
    (enc, xr), _ = run(x, weight)
    return enc, xr
